# revision 7
# baseline (speedup 1.0000x reference)
"""DynamicMaskHead Trainium2 kernel.

Per-instance 3-layer MLP over pixels (grouped 1x1 convs):
    out = w2 @ relu(w1 @ relu(w0 @ x + b0) + b1) + b2
with 128 instances, x: [10, 25600] per instance.

Sharding: 16 instances per NeuronCore (8 cores, data-parallel).

v2 design (per core, instances j in [0,16)):
  - L1: K=128 block-diagonal matmul over (j, cin 0..7) into PSUM, plus an
    accumulating K=32 matmul for cin 8..9 placed on PE row strip 32s
    (s = tile%4) via tile_position. Tiles are processed in GROUPs of 4 so
    the four K=32 matmuls sit on disjoint row strips and run concurrently
    (hardware sub-array tiling), costing ~1 matmul instead of 4.
  - L2: one K=128 block-diagonal matmul per tile.
  - L3: K=128 -> M=16 matmul per tile on PE column strip 32s; the four
    matmuls of a group run concurrently on disjoint column strips.
  - Epilogues: L1's bias+relu on ScalarE batched as FD=1024 over a
    2-bank PSUM pair tile; L2's bias+relu on VectorE (tensor_scalar
    add+max) per tile; L3's bias on ScalarE once per group. This splits
    the PSUM-evacuation work evenly across the two engines that can
    read PSUM.
  - All inputs are made SBUF-resident up front: x1 streams in 10
    contiguous chunk DMAs on the SP queue; x2 is PRE-PACKED ON THE HOST
    into its strip layout so it loads with 2 contiguous DMAs (no strided
    gather); weights load first on the GpSimd queue.
  - Output is staged in 2-group chunks and DMA'd continuously on the
    GpSimd (SWDGE) queue so no output drain remains at the end.
  - Matmul operands are bf16 (fp32 PSUM accumulate); host rounds to
    bf16 so the HBM stream is half-width.
"""

import sys

if "/opt/trn_rl_repo" not in sys.path:
    sys.path.insert(0, "/opt/trn_rl_repo")

import ml_dtypes
import numpy as np

N_CORES = 8
N_INST = 128
C_IN = 10
C = 8
H = W = 160
P = H * W          # 25600 pixels
PER = N_INST // N_CORES  # 16 instances per core
F = 512            # pixels per matmul tile (one fp32 PSUM bank)
NTILE = P // F     # 50
GROUP = 4          # tiles per pipeline group (one strip each)
NG = (NTILE + GROUP - 1) // GROUP  # 13 (last group has 2 tiles)
NSLOT = (NTILE + 3) // 4           # 13 x2 slots per strip
X2A_SLOTS = 7                      # x2 split into two resident tiles
CH = 2560          # x1 pixels per DMA chunk (5 tiles)
NCH = P // CH      # 10

_cached_nc = None


def _build():
    from concourse import bacc, bass, mybir, tile

    nc = bacc.Bacc("TRN2", target_bir_lowering=False, debug=False)
    f32 = mybir.dt.float32
    bf16 = mybir.dt.bfloat16
    Relu = mybir.ActivationFunctionType.Relu
    Ident = mybir.ActivationFunctionType.Identity
    op_add = mybir.AluOpType.add
    op_max = mybir.AluOpType.max

    x1_d = nc.dram_tensor("x1", [128, P], bf16, kind="ExternalInput")
    x2p_d = nc.dram_tensor("x2p", [128, NSLOT * F], bf16, kind="ExternalInput")
    w1a_d = nc.dram_tensor("w1a", [128, 128], bf16, kind="ExternalInput")
    w1b_d = nc.dram_tensor("w1b", [32, 128], bf16, kind="ExternalInput")
    w2_d = nc.dram_tensor("w2", [128, 128], bf16, kind="ExternalInput")
    w3_d = nc.dram_tensor("w3", [128, 16], bf16, kind="ExternalInput")
    b0_d = nc.dram_tensor("b0", [128, 1], f32, kind="ExternalInput")
    b1_d = nc.dram_tensor("b1", [128, 1], f32, kind="ExternalInput")
    b2r_d = nc.dram_tensor("b2r", [128, 1], f32, kind="ExternalInput")
    out_d = nc.dram_tensor("out", [PER, P], f32, kind="ExternalOutput")

    with tile.TileContext(nc) as tc:
        with (
            tc.tile_pool(name="const", bufs=1) as cpool,
            tc.tile_pool(name="hp", bufs=5) as hpool,
            tc.tile_pool(name="h2p", bufs=12) as h2pool,
            tc.tile_pool(name="op", bufs=2) as opool,
            tc.tile_pool(name="ps1", bufs=2, space="PSUM") as pp1,
            tc.tile_pool(name="ps2", bufs=3, space="PSUM") as pp2,
            tc.tile_pool(name="ps3", bufs=1, space="PSUM") as pp3,
        ):
            # ---- inputs split across the two HWDGE queues, ordered so the
            # first group's operands land first: SP gets w1a + the x1
            # stream; ACT gets w1b/x2/biases/other weights. ----
            w1a = cpool.tile([128, 128], bf16)
            nc.sync.dma_start(w1a[:], w1a_d[:])
            w1b = cpool.tile([128, 128], bf16)
            for k in range(4):
                nc.scalar.dma_start(w1b[32 * k : 32 * k + 32, :], w1b_d[:])
            x2a = cpool.tile([128, X2A_SLOTS * F], bf16, name="x2a")
            nc.scalar.dma_start(x2a[:], x2p_d[:, 0 : X2A_SLOTS * F])
            b0t = cpool.tile([128, 1], f32)
            nc.scalar.dma_start(b0t[:], b0_d[:])
            w2t = cpool.tile([128, 128], bf16)
            nc.scalar.dma_start(w2t[:], w2_d[:])
            w3t = cpool.tile([128, 16], bf16)
            nc.scalar.dma_start(w3t[:], w3_d[:])
            b1t = cpool.tile([128, 1], f32)
            nc.scalar.dma_start(b1t[:], b1_d[:])
            b2rt = cpool.tile([128, 1], f32)
            nc.scalar.dma_start(b2rt[:], b2r_d[:])
            x2b = cpool.tile([128, (NSLOT - X2A_SLOTS) * F], bf16, name="x2b")
            nc.scalar.dma_start(x2b[:], x2p_d[:, X2A_SLOTS * F :])

            # ---- x1 chunks on the SP queue, all prefetched up front ----
            x1c = []
            for c in range(NCH):
                xc = cpool.tile([128, CH], bf16, name=f"x1c{c}")
                nc.sync.dma_start(xc[:], x1_d[:, c * CH : (c + 1) * CH])
                x1c.append(xc)

            # PE warm-up: dummy matmuls on zeroed data while the first
            # x DMAs are in flight, so HAM un-throttles before real work.
            wdum = cpool.tile([128, 128], bf16, name="wdum")
            nc.vector.memset(wdum[:], 0.0)
            xdum = cpool.tile([128, F], bf16, name="xdum")
            nc.vector.memset(xdum[:], 0.0)
            for wi in range(8):
                psw = pp1.tile([128, F], f32, name="psw", tag="ps1")
                nc.tensor.matmul(
                    psw[:], wdum[:], xdum[:], start=True, stop=True
                )

            def x2col(t):
                s, q = t % 4, t // 4
                if q < X2A_SLOTS:
                    return x2a[32 * s : 32 * s + 32, bass.ts(q, F)], s
                return x2b[32 * s : 32 * s + 32, bass.ts(q - X2A_SLOTS, F)], s

            # 3-stage software pipeline over 4-tile groups.
            st = {}

            def tiles_of(g):
                return list(range(GROUP * g, min(GROUP * g + GROUP, NTILE)))

            def stage_l1(g):
                tiles = tiles_of(g)
                ps1s = {}
                for t in tiles:
                    ps1s[t] = pp1.tile([128, F], f32, name="ps1", tag="ps1")
                # L1a: full-array K=128 matmuls, back-to-back
                for t in tiles:
                    c, r = divmod(t, 5)
                    nc.tensor.matmul(
                        ps1s[t][:], w1a[:], x1c[c][:, bass.ts(r, F)],
                        start=True, stop=False,
                    )
                # L1b: K=32 strip matmuls on disjoint row strips,
                # back-to-back so they run concurrently on the PE
                for t in tiles:
                    rhs, s = x2col(t)
                    nc.tensor.matmul(
                        ps1s[t][:],
                        w1b[32 * s : 32 * s + 32, :],
                        rhs,
                        start=False, stop=True,
                        tile_position=(32 * s, 0),
                    )
                # bias+relu epilogue on ScalarE, one op per tile
                h1s = {}
                for t in tiles:
                    h1 = hpool.tile([128, F], bf16, name="h1", tag="h1")
                    nc.scalar.activation(h1[:], ps1s[t][:], Relu, bias=b0t[:])
                    h1s[t] = h1
                st[g] = {"h1s": h1s}

            def stage_l2(g):
                h1s = st[g]["h1s"]
                h2s = {}
                for t in sorted(h1s):
                    ps2 = pp2.tile([128, F], f32, name="ps2", tag="ps2")
                    nc.tensor.matmul(
                        ps2[:], w2t[:], h1s[t][:],
                        start=True, stop=True,
                    )
                    h2s[t] = (h2pool.tile([128, F], bf16, name="h2", tag="h2"), ps2)
                for t in sorted(h2s):
                    h2, ps2 = h2s[t]
                    nc.vector.tensor_scalar(
                        h2[:], ps2[:], b1t[:], 0.0, op0=op_add, op1=op_max
                    )
                st[g]["h2s"] = {t: v[0] for t, v in h2s.items()}

            def stage_l3(g):
                h2s = st[g]["h2s"]
                tiles = sorted(h2s)
                ps3 = pp3.tile([128, F], f32, name="ps3", tag="ps3")
                # four M=16 matmuls on disjoint column strips, concurrent
                for t in tiles:
                    s = t % 4
                    nc.tensor.matmul(
                        ps3[32 * s : 32 * s + 16, :], w3t[:], h2s[t][:],
                        start=True, stop=True,
                        tile_position=(0, 32 * s),
                    )
                oc = opool.tile([128, F], f32, name="oc", tag="out")
                nrows = 32 * (len(tiles) - 1) + 16
                nc.scalar.activation(
                    oc[0:nrows, :], ps3[0:nrows, :],
                    Ident, bias=b2rt[0:nrows, :],
                )
                # stream the group's output immediately, alternating
                # between the two otherwise-idle DMA queues
                for t in tiles:
                    s = t % 4
                    eng = nc.gpsimd if (g % 2) else nc.sync
                    eng.dma_start(
                        out_d[:, t * F : (t + 1) * F],
                        oc[32 * s : 32 * s + 16, :],
                    )

            for i in range(NG + 2):
                if i < NG:
                    stage_l1(i)
                if 0 <= i - 1 < NG:
                    stage_l2(i - 1)
                if 0 <= i - 2 < NG:
                    stage_l3(i - 2)
                    del st[i - 2]

    nc.compile()
    return nc


def _prep_inputs(features, params):
    feats = np.ascontiguousarray(features, dtype=np.float32).reshape(N_INST, C_IN, P)
    params = np.asarray(params, dtype=np.float32)
    bf = ml_dtypes.bfloat16
    in_maps = []
    for c in range(N_CORES):
        js = slice(c * PER, (c + 1) * PER)
        pc = params[js]
        w0 = pc[:, :80].reshape(PER, C, C_IN)
        w1 = pc[:, 80:144].reshape(PER, C, C)
        w2 = pc[:, 144:152].reshape(PER, 1, C)
        b0 = pc[:, 152:160]
        b1 = pc[:, 160:168]
        b2 = pc[:, 168:169]
        w1a = np.zeros((128, 128), np.float32)
        w1b = np.zeros((32, 128), np.float32)
        w2b = np.zeros((128, 128), np.float32)
        w3b = np.zeros((128, 16), np.float32)
        for j in range(PER):
            w1a[j * 8 : j * 8 + 8, j * 8 : j * 8 + 8] = w0[j, :, :8].T
            w1b[j * 2 : j * 2 + 2, j * 8 : j * 8 + 8] = w0[j, :, 8:10].T
            w2b[j * 8 : j * 8 + 8, j * 8 : j * 8 + 8] = w1[j].T
            w3b[j * 8 : j * 8 + 8, j] = w2[j, 0, :]
        b2rep = np.zeros((128, 1), np.float32)
        for k in range(4):
            b2rep[32 * k : 32 * k + 16, 0] = b2[:, 0]
        x = feats[js]
        # x2 strip packing: tile t (512 px) -> strip s=t%4 (partitions
        # 32s + 2j + cin), slot q=t//4 (columns [qF, (q+1)F)).
        x2src = x[:, 8:10, :].reshape(PER, 2, NTILE, F)
        x2p = np.zeros((128, NSLOT * F), np.float32)
        x2v = x2p.reshape(4, 32, NSLOT, F)
        for s in range(4):
            ts_ = list(range(s, NTILE, 4))
            x2v[s].reshape(PER, 2, NSLOT, F)[:, :, 0 : len(ts_), :] = (
                x2src[:, :, ts_, :]
            )
        in_maps.append(
            {
                "x1": np.ascontiguousarray(x[:, :8, :]).reshape(128, P).astype(bf),
                "x2p": x2p.astype(bf),
                "w1a": w1a.astype(bf),
                "w1b": w1b.astype(bf),
                "w2": w2b.astype(bf),
                "w3": w3b.astype(bf),
                "b0": np.ascontiguousarray(b0).reshape(128, 1),
                "b1": np.ascontiguousarray(b1).reshape(128, 1),
                "b2r": b2rep,
            }
        )
    return in_maps


def _run(features, params, trace=False, **kwargs):
    global _cached_nc
    from concourse.bass_utils import run_bass_kernel_spmd

    if _cached_nc is None:
        _cached_nc = _build()
    in_maps = _prep_inputs(features, params)
    res = run_bass_kernel_spmd(
        _cached_nc, in_maps, list(range(N_CORES)), trace=trace, **kwargs
    )
    out = np.empty((N_INST, 1, H, W), np.float32)
    for c in range(N_CORES):
        out[c * PER : (c + 1) * PER, 0] = res.results[c]["out"].reshape(PER, H, W)
    return out, res


def kernel(features, params, num_insts=None, **_ignored):
    out, _ = _run(features, params, trace=False)
    return out
